# revision 27
# baseline (speedup 1.0000x reference)
"""MemoryReader kernel for Trainium2, data-parallel over batch across 8 cores.

Per batch element b (one NeuronCore each):
    mkf = mk[b] as [CK=64, M=4096], qkf = qk[b] as [CK, N=4096]
    aff[m, n] = (2 * mkf.T @ qkf - |mkf[:,m]|^2) / sqrt(CK)
    P = softmax over m
    mem[c, n]  = sum_m mv[b][c, m] * P[m, n]
    out[b] = concat([mem, qv[b]], channel axis)

Device kernel layout (per core):
    - QK^T matmuls produce aff tiles in [m-partition, n-free] layout,
      32 m-chunks of [128, 512] per n-super-tile of 512 columns.
    - ScalarE computes E = exp(0.25*ab - a_sq/8) straight out of PSUM
      (per-partition bias = -a_sq/8; logits are bounded so the max
      subtraction of a standard softmax is unnecessary in fp32).
    - VectorE accumulates sum_m E chunk-by-chunk; a ones-vector matmul
      folds the partition axis; reciprocal + DMA partition-broadcast
      give 1/s replicated across partitions.
    - Readout matmuls contract over m in PSUM (4 c-chunks of 128), then
      VectorE scales by 1/s while evacuating PSUM.
    - mv^T / mk^T are prepared host-side (pure layout transforms), so no
      on-device transposes are needed. qv never touches the device.
"""

import os
import sys

import numpy as np

B, CK, CV, H, W = 8, 64, 512, 64, 64
M = H * W          # memory positions per batch element
N = H * W          # query positions
NT = 512           # n-super-tile width (columns per softmax pass)
NSUP = N // NT     # 8 n-super-tiles
MCH = M // 128     # 32 m-chunks
N_CORES = 8

# "fp32r" runs matmuls in relaxed-precision single-pass mode (4x faster
# than exact fp32 on the PE array); "fp32" is exact.
MATMUL_PREC = os.environ.get("KERNEL_MATMUL_PREC", "fp32r")

_CACHE = {}


def _build_program():
    sys.path.insert(0, "/opt/trn_rl_repo")
    from contextlib import ExitStack

    import concourse.tile as tile
    from concourse import bacc, mybir

    dt = mybir.dt
    f32 = dt.float32
    # Matmul operand dtype: float32r (relaxed single-pass fp32, 4x faster
    # on the PE array) or exact float32. Bit-layout is identical; walrus
    # requires producers of fp32r matmul operands to be typed fp32r.
    mdt = dt.float32r if MATMUL_PREC == "fp32r" else f32

    nc = bacc.Bacc("TRN2", target_bir_lowering=False, debug=False,
                   num_devices=N_CORES)

    mk_d = nc.dram_tensor("mk", [128, M], mdt, kind="ExternalInput").ap()
    mkt_d = nc.dram_tensor("mkt", [128, MCH * CK], f32,
                           kind="ExternalInput").ap()
    qk_d = nc.dram_tensor("qk", [128, N], mdt, kind="ExternalInput").ap()
    mvt_d = nc.dram_tensor("mvt", [MCH, 128, CV], mdt,
                           kind="ExternalInput").ap()
    mem_d = nc.dram_tensor("mem", [CV, N], f32, kind="ExternalOutput").ap()

    with tile.TileContext(nc) as tc, ExitStack() as ctx:
        sing = ctx.enter_context(tc.tile_pool(name="sing", bufs=1))
        e_pool = ctx.enter_context(tc.tile_pool(name="E", bufs=37))
        scratch = ctx.enter_context(tc.tile_pool(name="scratch", bufs=2))
        sacc_pool = ctx.enter_context(tc.tile_pool(name="sacc", bufs=2))
        row_pool = ctx.enter_context(tc.tile_pool(name="row", bufs=2))
        rb_pool = ctx.enter_context(tc.tile_pool(name="rb", bufs=2))
        out_pool = ctx.enter_context(tc.tile_pool(name="out", bufs=5))
        qk_ps_pool = ctx.enter_context(
            tc.tile_pool(name="qkps", bufs=3, space="PSUM"))
        ro_ps_pool = ctx.enter_context(
            tc.tile_pool(name="rops", bufs=1, space="PSUM"))
        misc_ps_pool = ctx.enter_context(
            tc.tile_pool(name="miscps", bufs=1, space="PSUM"))

        # PE warmup: the PE activity monitor starts throttled at 1.2 GHz
        # and needs ~3.4us of sustained matmul activity to unthrottle.
        # Burn dummy matmuls while the input DMAs stream so the real
        # matmuls start at 2.4 GHz.
        warm_sb = sing.tile([128, NT], f32)
        nc.vector.memset(warm_sb[:], 1.0)
        warm_ps = qk_ps_pool.tile([128, NT], f32, tag="qk_ps", name="warm_ps")
        for w in range(56):
            nc.tensor.matmul(warm_ps[:, 0:128], lhsT=warm_sb[:, 0:128],
                             rhs=warm_sb[:, 0:128], start=True, stop=True)

        # Resident inputs. mk/qk are zero-padded from CK=64 to K=128
        # contraction rows: K=64 matmuls leave the PE activity monitor
        # throttled at 1.2 GHz (measured 427 ns/MM vs 222 ns at K=128),
        # so padded K=128 matmuls are 2x faster despite wasting rows.
        # All DMAs go through the sync engine (hardware DGE); ordered so
        # the tensors gating the first matmuls arrive first.
        mk_sb = sing.tile([128, M], mdt)
        qk_sb = sing.tile([128, N], mdt)
        mkt_sb = sing.tile([128, MCH, CK], f32)
        mvt_sb = sing.tile([128, MCH, CV], mdt)
        for g in range(4):
            gs = slice(g * 1024, (g + 1) * 1024)
            nc.sync.dma_start(out=mk_sb[:, gs], in_=mk_d[:, gs])
        nc.sync.dma_start(out=qk_sb[:, 0:NT], in_=qk_d[:, 0:NT])
        nc.sync.dma_start(out=mkt_sb[:], in_=mkt_d[:].rearrange(
            "p (j c) -> p j c", c=CK))
        for j in range(4):
            nc.sync.dma_start(out=mvt_sb[:, j, :], in_=mvt_d[j])
        nc.sync.dma_start(out=qk_sb[:, NT:N], in_=qk_d[:, NT:N])
        for j in range(4, MCH):
            nc.sync.dma_start(out=mvt_sb[:, j, :], in_=mvt_d[j])

        # Ones vectors typed fp32r so the softmax-sum and broadcast
        # matmuls take the single-pass PE path (213 ns vs 853 ns).
        ones_f32 = sing.tile([128, 1], f32)
        nc.vector.memset(ones_f32[:], 1.0)
        ones_sb = sing.tile([128, 1], mdt)
        nc.vector.tensor_copy(ones_sb[:], ones_f32[:].bitcast(mdt))
        ones_row_f32 = sing.tile([1, 128], f32)
        nc.vector.memset(ones_row_f32[:], 1.0)
        ones_row = sing.tile([1, 128], mdt)
        nc.vector.tensor_copy(ones_row[:], ones_row_f32[:].bitcast(mdt))

        # Per-partition softmax bias: asq[p, j] = -|mk[:, j*128+p]|^2 / 8.
        # (tensor_tensor_reduce crashes on HW via this toolchain; use
        # Square -> free-axis reduce -> scale, in 4 pieces to keep the
        # scratch small.)
        asq = sing.tile([128, MCH], f32)
        for piece in range(4):
            js = slice(piece * 8, (piece + 1) * 8)
            sqp = scratch.tile([128, 8, CK], f32, tag="sqp",
                               name=f"sqp{piece}")
            nc.scalar.activation(sqp[:], mkt_sb[:, js, :],
                                 mybir.ActivationFunctionType.Square)
            nc.vector.tensor_reduce(asq[:, js], sqp[:],
                                    axis=mybir.AxisListType.X,
                                    op=mybir.AluOpType.add)
        nc.scalar.mul(asq[:], asq[:], -0.125)

        def emit_tail(ti, tsacc, tosbs, tnsl):
            # Softmax denominator, reciprocal, partition-broadcast and
            # final scaling for super `ti`. Emitted a few chunks into the
            # NEXT super so the PE stream has QK matmuls to chew on while
            # the DVE-side reduction chain resolves.
            s_ps = misc_ps_pool.tile([1, NT], f32, tag="misc",
                                     name=f"sps{ti}")
            nc.tensor.matmul(s_ps[:], lhsT=ones_sb[:], rhs=tsacc[:],
                             start=True, stop=True)
            s_row = row_pool.tile([1, NT], mdt, tag="srow",
                                  name=f"srow{ti}")
            with nc.allow_low_precision(reason="fp32r is fp32 bits"):
                nc.vector.reciprocal(s_row[:], s_ps[:].bitcast(mdt))
            rb_ps = misc_ps_pool.tile([128, NT], f32, tag="misc",
                                      name=f"rbps{ti}")
            nc.tensor.matmul(rb_ps[:], lhsT=ones_row[:], rhs=s_row[:],
                             start=True, stop=True)
            rb = rb_pool.tile([128, NT], f32, tag="rb", name=f"rb{ti}")
            nc.scalar.copy(rb[:], rb_ps[:])
            for c in range(4):
                nc.vector.tensor_mul(tosbs[c][:], tosbs[c][:], rb[:])
                nc.sync.dma_start(
                    out=mem_d[c * 128:(c + 1) * 128, tnsl], in_=tosbs[c][:])

        pending_tail = None
        for i in range(NSUP):
            nsl = slice(i * NT, (i + 1) * NT)
            ro_ps = [ro_ps_pool.tile([128, NT], f32, tag=f"ro{c}",
                                     name=f"ro{c}_{i}")
                     for c in range(4)]
            sacc = sacc_pool.tile([128, NT], mdt, tag="sacc",
                                  name=f"sacc{i}")
            for m in range(MCH):
                qk_ps = qk_ps_pool.tile([128, NT], f32, tag="qk_ps",
                                        name=f"qkps{i}_{m}")
                nc.tensor.matmul(
                    qk_ps[:],
                    lhsT=mk_sb[:, m * 128:(m + 1) * 128],
                    rhs=qk_sb[:, nsl],
                    start=True, stop=True)
                e = e_pool.tile([128, NT], mdt, tag="E", name=f"e{i}_{m}")
                nc.scalar.activation(
                    e[:], qk_ps[:], mybir.ActivationFunctionType.Exp,
                    bias=asq[:, m:m + 1], scale=0.25)
                # fp32r is bit-identical to fp32; the low-precision
                # gate only keys off the dtype tag.
                with nc.allow_low_precision(reason="fp32r is fp32 bits"):
                    if m == 0:
                        nc.vector.tensor_copy(sacc[:], e[:])
                    else:
                        nc.vector.tensor_add(sacc[:], sacc[:], e[:])
                if m == 4 and pending_tail is not None:
                    emit_tail(*pending_tail)
                    pending_tail = None
                for c in range(4):
                    nc.tensor.matmul(
                        ro_ps[c][:],
                        lhsT=mvt_sb[:, m, c * 128:(c + 1) * 128],
                        rhs=e[:],
                        start=(m == 0), stop=(m == MCH - 1))

            # Evacuate readout PSUM unscaled right away so the next
            # n-super's readout matmuls get their banks back without
            # waiting on the softmax-sum/reciprocal chain.
            osbs = []
            for c in range(4):
                osb = out_pool.tile([128, NT], f32, tag="osb",
                                    name=f"osb{i}_{c}")
                nc.vector.tensor_copy(osb[:], ro_ps[c][:])
                osbs.append(osb)
            pending_tail = (i, sacc, osbs, nsl)

        emit_tail(*pending_tail)

    nc.compile()
    return nc


def _get_program():
    if "nc" not in _CACHE:
        _CACHE["nc"] = _build_program()
    return _CACHE["nc"]


def _make_in_maps(mk, qk, mv):
    mk = np.asarray(mk, dtype=np.float32)
    qk = np.asarray(qk, dtype=np.float32)
    mv = np.asarray(mv, dtype=np.float32)
    in_maps = []
    zpad = np.zeros((128 - CK, M), dtype=np.float32)
    for b in range(B):
        mk_b = np.ascontiguousarray(
            np.concatenate([mk[b].reshape(CK, M), zpad], axis=0))
        qk_b = np.ascontiguousarray(
            np.concatenate([qk[b].reshape(CK, N), zpad], axis=0))
        # mkt[p, j*CK + c] = mk[b][c, j*128 + p]
        mkt_b = np.ascontiguousarray(
            mk[b].reshape(CK, MCH, 128).transpose(2, 1, 0).reshape(
                128, MCH * CK))
        # mvt[j, p, c] = mv[b][c, j*128 + p]
        mvt_b = np.ascontiguousarray(
            mv[b].reshape(CV, MCH, 128).transpose(1, 2, 0))
        in_maps.append({"mk": mk_b, "qk": qk_b, "mkt": mkt_b, "mvt": mvt_b})
    return in_maps


def kernel(mk, qk, mv, qv):
    qv = np.asarray(qv, dtype=np.float32)
    nc = _get_program()
    from concourse.bass_utils import run_bass_kernel_spmd

    in_maps = _make_in_maps(mk, qk, mv)
    res = run_bass_kernel_spmd(nc, in_maps, list(range(N_CORES)))
    mem = np.stack([res.results[b]["mem"] for b in range(B)], axis=0)
    mem = mem.reshape(B, CV, H, W)
    return np.concatenate([mem, qv], axis=1)


# revision 28
# speedup vs baseline: 1.1880x; 1.1880x over previous
"""MemoryReader kernel for Trainium2, data-parallel over batch across 8 cores.

Per batch element b (one NeuronCore each):
    mkf = mk[b] as [CK=64, M=4096], qkf = qk[b] as [CK, N=4096]
    aff[m, n] = (2 * mkf.T @ qkf - |mkf[:,m]|^2) / sqrt(CK)
    P = softmax over m
    mem[c, n]  = sum_m mv[b][c, m] * P[m, n]
    out[b] = concat([mem, qv[b]], channel axis)

Device kernel layout (per core):
    - QK^T matmuls produce aff tiles in [m-partition, n-free] layout,
      32 m-chunks of [128, 512] per n-super-tile of 512 columns.
    - ScalarE computes E = exp(0.25*ab - a_sq/8) straight out of PSUM
      (per-partition bias = -a_sq/8; logits are bounded so the max
      subtraction of a standard softmax is unnecessary in fp32).
    - VectorE accumulates sum_m E chunk-by-chunk; a ones-vector matmul
      folds the partition axis; reciprocal + DMA partition-broadcast
      give 1/s replicated across partitions.
    - Readout matmuls contract over m in PSUM (4 c-chunks of 128), then
      VectorE scales by 1/s while evacuating PSUM.
    - mv^T / mk^T are prepared host-side (pure layout transforms), so no
      on-device transposes are needed. qv never touches the device.
"""

import os
import sys

import numpy as np

B, CK, CV, H, W = 8, 64, 512, 64, 64
M = H * W          # memory positions per batch element
N = H * W          # query positions
NT = 512           # n-super-tile width (columns per softmax pass)
NSUP = N // NT     # 8 n-super-tiles
MCH = M // 128     # 32 m-chunks
N_CORES = 8

# "fp32r" runs matmuls in relaxed-precision single-pass mode (4x faster
# than exact fp32 on the PE array); "fp32" is exact.
MATMUL_PREC = os.environ.get("KERNEL_MATMUL_PREC", "fp32r")

_CACHE = {}


def _build_program():
    sys.path.insert(0, "/opt/trn_rl_repo")
    from contextlib import ExitStack

    import concourse.tile as tile
    from concourse import bacc, mybir

    dt = mybir.dt
    f32 = dt.float32
    # Matmul operand dtype: float32r (relaxed single-pass fp32, 4x faster
    # on the PE array) or exact float32. Bit-layout is identical; walrus
    # requires producers of fp32r matmul operands to be typed fp32r.
    mdt = dt.float32r if MATMUL_PREC == "fp32r" else f32

    nc = bacc.Bacc("TRN2", target_bir_lowering=False, debug=False,
                   num_devices=N_CORES)

    mk_d = nc.dram_tensor("mk", [128, M], mdt, kind="ExternalInput").ap()
    mkt_d = nc.dram_tensor("mkt", [128, MCH * CK], f32,
                           kind="ExternalInput").ap()
    qk_d = nc.dram_tensor("qk", [128, N], mdt, kind="ExternalInput").ap()
    mvt_d = nc.dram_tensor("mvt", [MCH, 128, CV], mdt,
                           kind="ExternalInput").ap()
    mem_d = nc.dram_tensor("mem", [CV, N], f32, kind="ExternalOutput").ap()

    with tile.TileContext(nc) as tc, ExitStack() as ctx:
        sing = ctx.enter_context(tc.tile_pool(name="sing", bufs=1))
        e_pool = ctx.enter_context(tc.tile_pool(name="E", bufs=34))
        scratch = ctx.enter_context(tc.tile_pool(name="scratch", bufs=2))
        sacc_pool = ctx.enter_context(tc.tile_pool(name="sacc", bufs=2))
        row_pool = ctx.enter_context(tc.tile_pool(name="row", bufs=2))
        rb_pool = ctx.enter_context(tc.tile_pool(name="rb", bufs=2))
        out_pool = ctx.enter_context(tc.tile_pool(name="out", bufs=8))
        qk_ps_pool = ctx.enter_context(
            tc.tile_pool(name="qkps", bufs=3, space="PSUM"))
        ro_ps_pool = ctx.enter_context(
            tc.tile_pool(name="rops", bufs=1, space="PSUM"))
        misc_ps_pool = ctx.enter_context(
            tc.tile_pool(name="miscps", bufs=1, space="PSUM"))

        # PE warmup: the PE activity monitor starts throttled at 1.2 GHz
        # and needs ~3.4us of sustained matmul activity to unthrottle.
        # Burn dummy matmuls while the input DMAs stream so the real
        # matmuls start at 2.4 GHz.
        warm_sb = sing.tile([128, NT], f32)
        nc.vector.memset(warm_sb[:], 1.0)
        warm_ps = qk_ps_pool.tile([128, NT], f32, tag="qk_ps", name="warm_ps")
        for w in range(56):
            nc.tensor.matmul(warm_ps[:, 0:128], lhsT=warm_sb[:, 0:128],
                             rhs=warm_sb[:, 0:128], start=True, stop=True)

        # Resident inputs. mk/qk are zero-padded from CK=64 to K=128
        # contraction rows: K=64 matmuls leave the PE activity monitor
        # throttled at 1.2 GHz (measured 427 ns/MM vs 222 ns at K=128),
        # so padded K=128 matmuls are 2x faster despite wasting rows.
        # All DMAs go through the sync engine (hardware DGE); ordered so
        # the tensors gating the first matmuls arrive first.
        mk_sb = sing.tile([128, M], mdt)
        qk_sb = sing.tile([128, N], mdt)
        mkt_sb = sing.tile([128, MCH, CK], f32)
        mvt_sb = sing.tile([128, MCH, CV], mdt)
        for g in range(4):
            gs = slice(g * 1024, (g + 1) * 1024)
            nc.sync.dma_start(out=mk_sb[:, gs], in_=mk_d[:, gs])
        nc.sync.dma_start(out=qk_sb[:, 0:NT], in_=qk_d[:, 0:NT])
        nc.sync.dma_start(out=mkt_sb[:], in_=mkt_d[:].rearrange(
            "p (j c) -> p j c", c=CK))
        for j in range(4):
            nc.sync.dma_start(out=mvt_sb[:, j, :], in_=mvt_d[j])
        nc.sync.dma_start(out=qk_sb[:, NT:N], in_=qk_d[:, NT:N])
        for j in range(4, MCH):
            nc.sync.dma_start(out=mvt_sb[:, j, :], in_=mvt_d[j])

        # Ones vectors typed fp32r so the softmax-sum and broadcast
        # matmuls take the single-pass PE path (213 ns vs 853 ns).
        ones_f32 = sing.tile([128, 1], f32)
        nc.vector.memset(ones_f32[:], 1.0)
        ones_sb = sing.tile([128, 1], mdt)
        nc.vector.tensor_copy(ones_sb[:], ones_f32[:].bitcast(mdt))
        ones_row_f32 = sing.tile([1, 128], f32)
        nc.vector.memset(ones_row_f32[:], 1.0)
        ones_row = sing.tile([1, 128], mdt)
        nc.vector.tensor_copy(ones_row[:], ones_row_f32[:].bitcast(mdt))

        # Per-partition softmax bias: asq[p, j] = -|mk[:, j*128+p]|^2 / 8.
        # (tensor_tensor_reduce crashes on HW via this toolchain; use
        # Square -> free-axis reduce -> scale, in 4 pieces to keep the
        # scratch small.)
        asq = sing.tile([128, MCH], f32)
        for piece in range(4):
            js = slice(piece * 8, (piece + 1) * 8)
            sqp = scratch.tile([128, 8, CK], f32, tag="sqp",
                               name=f"sqp{piece}")
            nc.scalar.activation(sqp[:], mkt_sb[:, js, :],
                                 mybir.ActivationFunctionType.Square)
            nc.vector.tensor_reduce(asq[:, js], sqp[:],
                                    axis=mybir.AxisListType.X,
                                    op=mybir.AluOpType.add)
        nc.scalar.mul(asq[:], asq[:], -0.125)

        def emit_tail(ti, tsacc, tosbs, tnsl):
            # Softmax denominator, reciprocal, partition-broadcast and
            # final scaling for super `ti`. Emitted a few chunks into the
            # NEXT super so the PE stream has QK matmuls to chew on while
            # the DVE-side reduction chain resolves.
            s_ps = misc_ps_pool.tile([1, NT], f32, tag="misc",
                                     name=f"sps{ti}")
            nc.tensor.matmul(s_ps[:], lhsT=ones_sb[:], rhs=tsacc[:],
                             start=True, stop=True)
            s_row = row_pool.tile([1, NT], mdt, tag="srow",
                                  name=f"srow{ti}")
            with nc.allow_low_precision(reason="fp32r is fp32 bits"):
                nc.vector.reciprocal(s_row[:], s_ps[:].bitcast(mdt))
            rb_ps = misc_ps_pool.tile([128, NT], f32, tag="misc",
                                      name=f"rbps{ti}")
            nc.tensor.matmul(rb_ps[:], lhsT=ones_row[:], rhs=s_row[:],
                             start=True, stop=True)
            rb = rb_pool.tile([128, NT], f32, tag="rb", name=f"rb{ti}")
            nc.scalar.copy(rb[:], rb_ps[:])
            for c in range(4):
                nc.vector.tensor_mul(tosbs[c][:], tosbs[c][:], rb[:])
                nc.sync.dma_start(
                    out=mem_d[c * 128:(c + 1) * 128, tnsl], in_=tosbs[c][:])

        pending_tail = None
        for i in range(NSUP):
            nsl = slice(i * NT, (i + 1) * NT)
            ro_ps = [ro_ps_pool.tile([128, NT], f32, tag=f"ro{c}",
                                     name=f"ro{c}_{i}")
                     for c in range(4)]
            sacc = sacc_pool.tile([128, NT], mdt, tag="sacc",
                                  name=f"sacc{i}")
            for m in range(MCH):
                qk_ps = qk_ps_pool.tile([128, NT], f32, tag="qk_ps",
                                        name=f"qkps{i}_{m}")
                nc.tensor.matmul(
                    qk_ps[:],
                    lhsT=mk_sb[:, m * 128:(m + 1) * 128],
                    rhs=qk_sb[:, nsl],
                    start=True, stop=True)
                e = e_pool.tile([128, NT], mdt, tag="E", name=f"e{i}_{m}")
                nc.scalar.activation(
                    e[:], qk_ps[:], mybir.ActivationFunctionType.Exp,
                    bias=asq[:, m:m + 1], scale=0.25)
                # fp32r is bit-identical to fp32; the low-precision
                # gate only keys off the dtype tag.
                with nc.allow_low_precision(reason="fp32r is fp32 bits"):
                    if m == 0:
                        nc.vector.tensor_copy(sacc[:], e[:])
                    else:
                        nc.vector.tensor_add(sacc[:], sacc[:], e[:])
                if m == 4 and pending_tail is not None:
                    emit_tail(*pending_tail)
                    pending_tail = None
                for c in range(4):
                    nc.tensor.matmul(
                        ro_ps[c][:],
                        lhsT=mvt_sb[:, m, c * 128:(c + 1) * 128],
                        rhs=e[:],
                        start=(m == 0), stop=(m == MCH - 1))

            # Evacuate readout PSUM unscaled right away so the next
            # n-super's readout matmuls get their banks back without
            # waiting on the softmax-sum/reciprocal chain.
            osbs = []
            for c in range(4):
                osb = out_pool.tile([128, NT], f32, tag="osb",
                                    name=f"osb{i}_{c}")
                nc.vector.tensor_copy(osb[:], ro_ps[c][:])
                osbs.append(osb)
            pending_tail = (i, sacc, osbs, nsl)

        emit_tail(*pending_tail)

    nc.compile()
    return nc


def _get_program():
    if "nc" not in _CACHE:
        _CACHE["nc"] = _build_program()
    return _CACHE["nc"]


def _make_in_maps(mk, qk, mv):
    mk = np.asarray(mk, dtype=np.float32)
    qk = np.asarray(qk, dtype=np.float32)
    mv = np.asarray(mv, dtype=np.float32)
    in_maps = []
    zpad = np.zeros((128 - CK, M), dtype=np.float32)
    for b in range(B):
        mk_b = np.ascontiguousarray(
            np.concatenate([mk[b].reshape(CK, M), zpad], axis=0))
        qk_b = np.ascontiguousarray(
            np.concatenate([qk[b].reshape(CK, N), zpad], axis=0))
        # mkt[p, j*CK + c] = mk[b][c, j*128 + p]
        mkt_b = np.ascontiguousarray(
            mk[b].reshape(CK, MCH, 128).transpose(2, 1, 0).reshape(
                128, MCH * CK))
        # mvt[j, p, c] = mv[b][c, j*128 + p]
        mvt_b = np.ascontiguousarray(
            mv[b].reshape(CV, MCH, 128).transpose(1, 2, 0))
        in_maps.append({"mk": mk_b, "qk": qk_b, "mkt": mkt_b, "mvt": mvt_b})
    return in_maps


def kernel(mk, qk, mv, qv):
    qv = np.asarray(qv, dtype=np.float32)
    nc = _get_program()
    from concourse.bass_utils import run_bass_kernel_spmd

    in_maps = _make_in_maps(mk, qk, mv)
    res = run_bass_kernel_spmd(nc, in_maps, list(range(N_CORES)))
    mem = np.stack([res.results[b]["mem"] for b in range(B)], axis=0)
    mem = mem.reshape(B, CV, H, W)
    return np.concatenate([mem, qv], axis=1)
